# revision 1
# baseline (speedup 1.0000x reference)
"""Causal self-attention (B=4, T=2048, D=1024, H=16) for Trainium2, 8-core SPMD.

Sharding: core c handles batch b = c//2 and heads [8*(c%2), 8*(c%2)+8).
Each core computes its 8 heads' Q/K/V projections plus flash-style causal
attention, writing out[b, :, 512*(c%2) : 512*(c%2)+512].

Per-core device program (all matmuls via the PE array in float32r):
  phase 0: transpose hs[b] -> hsT (PE transpose), transpose W slices -> WT
  phase 1: QT = WqT.T @ hsT (per-head-dim-major), KT likewise, V natural
  phase 2: per (head, q-chunk): sT = K.T-major scores, exp on ACT (scale and
           padding mask folded in), P.T@V via one matmul with a ones column
           appended to V to accumulate the softmax denominator, transpose the
           [65, q] result back via PE, normalize with reciprocal on DVE.
"""

import numpy as np

B, T, D, H = 4, 2048, 1024, 16
HD = D // H            # 64
N_CORES = 8
H_PER_CORE = H // 2    # 8
DOUT = H_PER_CORE * HD # 512

MASK_VAL = -1e10

_BUILT = {}


def _split_pe_waits(nc):
    """Walrus allows only one sync wait on fp32/f32r Matmult and on static
    DMACopy instructions.

    Hoist excess waits onto standalone EventSemaphore instructions placed
    immediately before on the same engine sequencer (semantics unchanged).
    """
    from concourse import mybir

    n_split = 0
    fn = nc.m.functions[0]
    for blk in fn.blocks:
        out = []
        for inst in blk.instructions:
            si = inst.sync_info
            cap = 1
            if si is not None and len(si.on_wait) > cap:
                waits = list(si.on_wait)
                for k, w in enumerate(waits[:-cap]):
                    ev = mybir.InstEventSemaphore(
                        name=f"{inst.name}-hw{k}",
                        ins=[],
                        outs=[],
                        engine=inst.engine,
                        sync_info=mybir.SyncInfo(on_wait=[w], on_update=[]),
                    )
                    nc.register_instruction(ev)
                    out.append(ev)
                inst.sync_info = mybir.SyncInfo(
                    on_wait=waits[-cap:], on_update=list(si.on_update)
                )
                n_split += 1
            out.append(inst)
        blk.instructions = out
    return n_split


def _build(T, DIN, NH, hd, has_qk_bias=False, has_v_bias=False, tune=None):
    """Build the single-core SPMD Bass program. Returns (nc, names)."""
    from contextlib import ExitStack

    import concourse.bass as bass
    import concourse.tile as tile
    from concourse import mybir
    from concourse.masks import make_identity

    f32 = mybir.dt.float32
    f32r = mybir.dt.float32r
    AF = mybir.ActivationFunctionType

    DOUT = NH * hd
    NT = T // 128            # k/q tiles
    KD = DIN // 128          # contraction tiles
    TCH = 512                # q-chunk width (one PSUM bank of fp32)
    NQC = T // TCH
    VW = hd + 1              # V columns + ones column for the denominator
    DCH = min(512, DIN)      # d-chunk width for streaming loads
    NDCH = DIN // DCH
    PT_OUT = DOUT // 128     # partition tiles of QT/KT
    scale = 1.0 / float(np.sqrt(hd))
    tune = dict(tune or {})
    MM_BUFS = tune.get("mm_bufs", 4)
    TP_BUFS = tune.get("tp_bufs", 2)
    OT_BUFS = tune.get("oT_bufs", 2)
    E_BUFS = tune.get("e_bufs", 4)
    OTSB_BUFS = tune.get("oTsb_bufs", 3)
    STREAM_BUFS = tune.get("stream_bufs", 3)
    SMALL_BUFS = tune.get("small_bufs", 4)
    WCH_CFG = tune.get("wch", 1024)
    TWO_PASS = tune.get("two_pass", False)
    PHASES = tune.get("phases", (1, 2))

    assert T % 512 == 0 and DIN % 128 == 0 and DOUT % 128 == 0 and hd == 64

    nc = bass.Bass("TRN2", target_bir_lowering=False, debug=False)

    hs_d = nc.dram_tensor("hs", [T, DIN], f32, kind="ExternalInput").ap()
    w_d = {
        k: nc.dram_tensor(f"w{k}", [DOUT, DIN], f32, kind="ExternalInput").ap()
        for k in "qkv"
    }
    b_d = {
        k: nc.dram_tensor(f"b{k}", [DOUT], f32, kind="ExternalInput").ap()
        for k in "qkv"
    }
    mask_d = nc.dram_tensor("mask", [T], f32, kind="ExternalInput").ap()
    out_d = nc.dram_tensor("out", [T, DOUT], f32, kind="ExternalOutput").ap()

    def copy_ps(dst, src, i):
        # phase-1-only helper: alternate PSUM->SBUF copies between DVE and
        # ACT (ACT is idle until attention starts)
        if i % 2 == 0:
            nc.vector.tensor_copy(out=dst, in_=src)
        else:
            nc.scalar.activation(out=dst, in_=src, func=AF.Copy)

    with tile.TileContext(nc) as tc, ExitStack() as ctx:
        persist = ctx.enter_context(tc.tile_pool(name="persist", bufs=1))
        ps_mm = ctx.enter_context(tc.tile_pool(name="ps_mm", bufs=MM_BUFS, space="PSUM"))
        ps_tp = ctx.enter_context(tc.tile_pool(name="ps_tp", bufs=TP_BUFS, space="PSUM"))

        ident = persist.tile([128, 128], f32, tag="ident", name="ident")
        make_identity(nc, ident)

        # padding mask, column kt holds mask[128*kt + p] across partitions p
        maskc = persist.tile([128, NT], f32, tag="maskc", name="maskc")
        nc.sync.dma_start(out=maskc, in_=mask_d.rearrange("(c p) -> p c", p=128))

        ones_col = persist.tile([128, NH, 1], f32, tag="ones_col", name="ones_col")
        nc.vector.memset(ones_col, 1.0)
        KT = [persist.tile([128, T], f32r, tag=f"kt{i}", name=f"kt{i}") for i in range(PT_OUT)]
        V = [persist.tile([128, NH * VW], f32r, tag=f"v{i}", name=f"v{i}") for i in range(NT)]

        bias_sb = {}
        if has_qk_bias:
            for k in ("q", "k"):
                bias_sb[k] = persist.tile([128, PT_OUT], f32, tag=f"bias{k}", name=f"bias{k}")
                nc.sync.dma_start(
                    out=bias_sb[k], in_=b_d[k].rearrange("(c p) -> p c", p=128)
                )
        bv_bc = None
        if has_v_bias:
            ones_row = persist.tile([1, 128], f32, tag="ones_row", name="ones_row")
            nc.vector.memset(ones_row, 1.0)
            bv_stage = persist.tile([1, DOUT], f32, tag="bv_stage", name="bv_stage")
            nc.sync.dma_start(out=bv_stage, in_=b_d["v"][None, :])
            bv_bc = persist.tile([128, DOUT], f32, tag="bv_bc", name="bv_bc")
            bps = ps_mm.tile([128, DOUT], f32, tag="mm", name="bps")
            nc.tensor.matmul(bps, lhsT=ones_row, rhs=bv_stage, start=True, stop=True)
            nc.vector.tensor_copy(out=bv_bc, in_=bps)

        # ------- chunk-fused pipeline: per 512-column T-chunk do -------
        #   hs load -> PE transpose -> V projection -> Q/K projection ->
        #   causal attention for the q-window of this chunk (all heads).
        # Attention for window c needs K/V chunks 0..c only, so it starts
        # right after the first chunk instead of after all projections.
        # QT is consumed only by its own window, so it is chunk-transient.
        WCH = TCH
        HPW = 1
        hsT_pool = ctx.enter_context(
            tc.tile_pool(name="hsT", bufs=tune.get("hsT_chunks", 2) * KD)
        )
        qtc_pool = ctx.enter_context(
            tc.tile_pool(
                name="qtc",
                bufs=(NQC if TWO_PASS else tune.get("qtc_chunks", 2)) * PT_OUT,
            )
        )
        all_QTc = []
        stream = ctx.enter_context(tc.tile_pool(name="stream", bufs=STREAM_BUFS))
        epool = ctx.enter_context(tc.tile_pool(name="epool", bufs=E_BUFS))
        oTsb_pool = ctx.enter_context(tc.tile_pool(name="oTsb", bufs=OTSB_BUFS))
        small = ctx.enter_context(tc.tile_pool(name="small", bufs=SMALL_BUFS))
        ps_oT = ctx.enter_context(
            tc.tile_pool(name="ps_oT", bufs=OT_BUFS, space="PSUM")
        )
        wt_pool = ctx.enter_context(tc.tile_pool(name="wt", bufs=KD))

        ci = 0

        # all three weight matrices transposed up-front
        wt = {}
        for key in ("v", "q", "k"):
            wt[key] = [
                wt_pool.tile([128, DOUT], f32r, tag=f"wt{key}", name=f"wt{key}")
                for _ in range(KD)
            ]
            for pt in range(PT_OUT if 1 in PHASES else 0):
                for dc in range(NDCH):
                    wn = stream.tile([128, DCH], f32, tag="st", name="wn")
                    nc.sync.dma_start(
                        out=wn,
                        in_=w_d[key][
                            128 * pt : 128 * (pt + 1), DCH * dc : DCH * (dc + 1)
                        ],
                    )
                    for j in range(DCH // 128):
                        kd = dc * (DCH // 128) + j
                        tp = ps_tp.tile([128, 128], f32, tag="tp", name="tp")
                        nc.tensor.transpose(tp, wn[:, 128 * j : 128 * (j + 1)], ident)
                        copy_ps(wt[key][kd][:, 128 * pt : 128 * (pt + 1)], tp, ci)
                        ci += 1

        def attn_window(h, c, QTc):
            pt_h = (hd * h) // 128
            off = (hd * h) % 128
            w0 = TCH * c
            oT = ps_oT.tile([VW, TCH], f32, tag="oT", name="oT")
            last_kt = (c + 1) * (TCH // 128) - 1
            for kt in range(last_kt + 1):
                qs = max(w0, 128 * kt)
                off2 = qs - w0
                n = TCH - off2
                ps = ps_mm.tile([128, TCH], f32, tag="mm", name="ps")
                nc.tensor.matmul(
                    ps[:, off2:TCH],
                    lhsT=KT[pt_h][off : off + hd, 128 * kt : 128 * (kt + 1)],
                    rhs=QTc[pt_h][off : off + hd, off2:TCH],
                    start=True,
                    stop=True,
                )
                e = epool.tile([128, n], f32r, tag="e", name="e")
                nc.scalar.activation(
                    out=e,
                    in_=ps[:, off2:TCH],
                    func=AF.Exp,
                    bias=maskc[:, kt : kt + 1],
                    scale=scale,
                )
                if 128 * kt >= w0:
                    # zero the upper-triangular (k > q) part of the causal
                    # diagonal block on the otherwise-idle GPSIMD
                    nc.gpsimd.affine_select(
                        out=e[:, 0:128],
                        in_=e[:, 0:128],
                        compare_op=mybir.AluOpType.is_ge,
                        fill=0.0,
                        base=0,
                        pattern=[[1, 128]],
                        channel_multiplier=-1,
                    )
                nc.tensor.matmul(
                    oT[:, off2:TCH],
                    lhsT=V[kt][:, VW * h : VW * (h + 1)],
                    rhs=e,
                    start=(kt == 0),
                    stop=(kt == last_kt),
                )
            oT_sb = oTsb_pool.tile([VW, TCH], f32, tag="oTsb", name="oTsb")
            nc.vector.tensor_copy(out=oT_sb, in_=oT)
            for j in range(TCH // 128):
                tp = ps_tp.tile([128, VW], f32, tag="tp", name="tpo")
                nc.tensor.transpose(
                    tp, oT_sb[:, 128 * j : 128 * (j + 1)], ident[0:VW, 0:VW]
                )
                rcp = small.tile([128, 1], f32, tag="rcp", name="rcp")
                nc.vector.reciprocal(rcp, tp[:, hd : hd + 1])
                ob = small.tile([128, hd], f32, tag="ob", name="ob")
                nc.vector.tensor_scalar_mul(ob, tp[:, 0:hd], rcp)
                nc.sync.dma_start(
                    out=out_d[
                        TCH * c + 128 * j : TCH * c + 128 * (j + 1),
                        hd * h : hd * (h + 1),
                    ],
                    in_=ob,
                )

        for c in range(NQC if 1 in PHASES else 0):
            hsTc = [
                hsT_pool.tile([128, TCH], f32r, tag="hsT", name="hsT")
                for _ in range(KD)
            ]
            # hs load + transpose + V projection for the 4 t-tiles of chunk c
            for tj in range(TCH // 128):
                tt = c * (TCH // 128) + tj
                for dc in range(NDCH):
                    hsn = stream.tile([128, DCH], f32, tag="st", name="hsn")
                    nc.sync.dma_start(
                        out=hsn,
                        in_=hs_d[128 * tt : 128 * (tt + 1), DCH * dc : DCH * (dc + 1)],
                    )
                    for j in range(DCH // 128):
                        kd = dc * (DCH // 128) + j
                        tp = ps_tp.tile([128, 128], f32, tag="tp", name="tp")
                        nc.tensor.transpose(tp, hsn[:, 128 * j : 128 * (j + 1)], ident)
                        copy_ps(hsTc[kd][:, 128 * tj : 128 * (tj + 1)], tp, ci)
                        ci += 1
                ps = ps_mm.tile([128, DOUT], f32, tag="mm", name="ps")
                for kd in range(KD):
                    nc.tensor.matmul(
                        ps,
                        lhsT=hsTc[kd][:, 128 * tj : 128 * (tj + 1)],
                        rhs=wt["v"][kd],
                        start=(kd == 0),
                        stop=(kd == KD - 1),
                    )
                vv = V[tt].rearrange("p (h x) -> p h x", x=VW)
                nc.vector.tensor_copy(
                    out=vv[:, :, 0:hd],
                    in_=ps.rearrange("p (h x) -> p h x", x=hd),
                )
                nc.vector.tensor_copy(out=vv[:, :, hd : hd + 1], in_=ones_col)
                if has_v_bias:
                    nc.vector.tensor_add(
                        vv[:, :, 0:hd],
                        vv[:, :, 0:hd],
                        bv_bc.rearrange("p (h x) -> p h x", x=hd),
                    )

            # Q (chunk-transient) and K (persistent) projections for chunk c
            QTc = [
                qtc_pool.tile([128, TCH], f32r, tag="qtc", name="qtc")
                for _ in range(PT_OUT)
            ]
            for pt in range(PT_OUT):
                for key in ("q", "k"):
                    ps = ps_mm.tile([128, TCH], f32, tag="mm", name="ps")
                    for kd in range(KD):
                        nc.tensor.matmul(
                            ps,
                            lhsT=wt[key][kd][:, 128 * pt : 128 * (pt + 1)],
                            rhs=hsTc[kd],
                            start=(kd == 0),
                            stop=(kd == KD - 1),
                        )
                    d_ap = (
                        QTc[pt][:]
                        if key == "q"
                        else KT[pt][:, TCH * c : TCH * (c + 1)]
                    )
                    if has_qk_bias:
                        nc.scalar.activation(
                            out=d_ap,
                            in_=ps,
                            func=AF.Identity,
                            bias=bias_sb[key][:, pt : pt + 1],
                        )
                    else:
                        copy_ps(d_ap, ps, ci)
                        ci += 1

            if 2 in PHASES and not TWO_PASS:
                for h in range(NH):
                    attn_window(h, c, QTc)
            if TWO_PASS:
                all_QTc.append(QTc)

        if TWO_PASS and 2 in PHASES:
            for c in range(NQC):
                for h in range(NH):
                    attn_window(h, c, all_QTc[c])


    _split_pe_waits(nc)
    return nc


class _Runner:
    """Build-once, run-many executor for an SPMD Bass program over N cores.

    Mirrors concourse.bass2jax.run_bass_via_pjrt but caches the jitted
    callable so repeated kernel() calls don't re-lower, and skips donation
    (this kernel writes every output element).
    """

    def __init__(self, nc, n_cores):
        import jax
        import jax.numpy as jnp  # noqa: F401
        from jax.experimental.shard_map import shard_map
        from jax.sharding import Mesh, PartitionSpec

        from concourse import mybir
        from concourse.bass2jax import (
            _bass_exec_p,
            install_neuronx_cc_hook,
            partition_id_tensor,
        )

        install_neuronx_cc_hook()
        self.nc = nc
        self.n_cores = n_cores
        partition_name = (
            nc.partition_id_tensor.name if nc.partition_id_tensor else None
        )

        in_names, out_names, out_avals, zero_outs = [], [], [], []
        for alloc in nc.m.functions[0].allocations:
            if not isinstance(alloc, mybir.MemoryLocationSet):
                continue
            name = alloc.memorylocations[0].name
            if alloc.kind == "ExternalInput":
                if name == partition_name:
                    continue
                in_names.append(name)
            elif alloc.kind == "ExternalOutput":
                out_names.append(name)
                shape = tuple(alloc.tensor_shape)
                dtype = mybir.dt.np(alloc.dtype)
                out_avals.append(jax.core.ShapedArray(shape, dtype))
                zero_outs.append(np.zeros(shape, dtype))
        self.in_names = list(in_names)
        self.out_names = list(out_names)
        self.out_avals = out_avals
        self.zero_outs = zero_outs
        n_params = len(in_names)
        all_in_names = in_names + out_names
        if partition_name is not None:
            all_in_names = all_in_names + [partition_name]

        def _body(*args):
            operands = list(args)
            if partition_name is not None:
                operands.append(partition_id_tensor())
            outs = _bass_exec_p.bind(
                *operands,
                out_avals=tuple(out_avals),
                in_names=tuple(all_in_names),
                out_names=tuple(out_names),
                lowering_input_output_aliases=(),
                sim_require_finite=True,
                sim_require_nnan=True,
                nc=nc,
            )
            return tuple(outs)

        devices = jax.devices()[:n_cores]
        assert len(devices) == n_cores
        mesh = Mesh(np.asarray(devices), ("core",))
        n_all = n_params + len(out_names)
        self._fn = jax.jit(
            shard_map(
                _body,
                mesh=mesh,
                in_specs=(PartitionSpec("core"),) * n_all,
                out_specs=(PartitionSpec("core"),) * len(out_names),
                check_rep=False,
            ),
            keep_unused=True,
        )
        self._jax = jax

    def prepare(self, in_maps):
        """Concatenate per-core inputs along axis 0 and device_put them."""
        jax = self._jax
        concat_in = [
            np.concatenate([np.asarray(m[name]) for m in in_maps], axis=0)
            for name in self.in_names
        ]
        concat_zero = [
            np.zeros((self.n_cores * z.shape[0], *z.shape[1:]), z.dtype)
            for z in self.zero_outs
        ]
        return [jax.device_put(a) for a in concat_in + concat_zero]

    def run_device(self, dev_args):
        out = self._fn(*dev_args)
        self._jax.block_until_ready(out)
        return out

    def __call__(self, in_maps):
        out_arrs = self.run_device(self.prepare(in_maps))
        res = []
        for c in range(self.n_cores):
            res.append(
                {
                    name: np.asarray(out_arrs[i]).reshape(
                        self.n_cores, *self.out_avals[i].shape
                    )[c]
                    for i, name in enumerate(self.out_names)
                }
            )
        return res


def _get_runner(has_qk_bias, has_v_bias):
    key = ("full", has_qk_bias, has_v_bias)
    if key not in _BUILT:
        nc = _build(T, D, H_PER_CORE, HD, has_qk_bias, has_v_bias)
        _BUILT[key] = _Runner(nc, N_CORES)
    return _BUILT[key]


def make_in_maps(hidden_states, attention_mask, Wq, bq, Wk, bk, Wv, bv):
    hs = np.ascontiguousarray(np.asarray(hidden_states, dtype=np.float32))
    am = np.asarray(attention_mask, dtype=np.float32).reshape(B, T)
    ws = {k: np.ascontiguousarray(np.asarray(w, dtype=np.float32)) for k, w in
          (("q", Wq), ("k", Wk), ("v", Wv))}
    bs = {k: np.asarray(b, dtype=np.float32) for k, b in
          (("q", bq), ("k", bk), ("v", bv))}
    in_maps = []
    for c in range(N_CORES):
        b = c // 2
        half = c % 2
        sl = slice(DOUT * half, DOUT * (half + 1))
        in_maps.append(
            {
                "hs": hs[b],
                "wq": ws["q"][sl],
                "wk": ws["k"][sl],
                "wv": ws["v"][sl],
                "bq": bs["q"][sl],
                "bk": bs["k"][sl],
                "bv": bs["v"][sl],
                "mask": am[b],
            }
        )
    return in_maps


def kernel(**inputs):
    hidden_states = np.asarray(inputs["hidden_states"], dtype=np.float32)
    attention_mask = np.asarray(inputs["attention_mask"], dtype=np.float32)
    Wq = np.asarray(inputs["Wq"], dtype=np.float32)
    bq = np.asarray(inputs["bq"], dtype=np.float32)
    Wk = np.asarray(inputs["Wk"], dtype=np.float32)
    bk = np.asarray(inputs["bk"], dtype=np.float32)
    Wv = np.asarray(inputs["Wv"], dtype=np.float32)
    bv = np.asarray(inputs["bv"], dtype=np.float32)

    has_qk_bias = bool(np.any(bq) or np.any(bk))
    has_v_bias = bool(np.any(bv))

    runner = _get_runner(has_qk_bias, has_v_bias)
    in_maps = make_in_maps(hidden_states, attention_mask, Wq, bq, Wk, bk, Wv, bv)
    res = runner(in_maps)

    out = np.empty((B, T, D), dtype=np.float32)
    for c in range(N_CORES):
        b = c // 2
        half = c % 2
        out[b, :, DOUT * half : DOUT * (half + 1)] = res[c]["out"]
    return out

